# revision 13
# baseline (speedup 1.0000x reference)
"""ALNN layer kernel for 8 TRN2 NeuronCores (raw Bass, explicit semaphores).

out[b,r,d] = relu( sum_l w_v[r,l,d]*relu(z[b,r,l,d]) + L*b_v[r,d] )
z = wt0*X + wt1*relu(X)*k + wt2*M + wt3*PD + 4*bt
k = exp(-relu(alpha_r)*|T - s_r|)        (uses relu(X*k) == relu(X)*k, k>0)

Sharding: B split 2 ways x R dealt into 4 buckets -> 8 cores, 16 b x 12 r
each. Cores c and c+4 share bucket c%4. r's with relu(alpha)=0 take a
fast path (k == 1: ABS/EXP/g skipped, q = relu(X)*wt1). The graph is
SPMD-uniform: a per-iteration fast-mask shared by all cores, sized by the
min zero-count across buckets (extra zeros run the slow path, still
correct since exp(-0*dist) == 1). Fast slots sit at the start (early DVE
start without waiting for ACT's k) and the last two slots (shorter drain).

Per-core layout: partitions = L(128), free = (b=16, d=64) = 1024.

v6 engine plan (v5 measured 59563ns: DVE busy 40.3us with 3.2us gaps,
PE busy 44.7us co-critical, fixed ~6.5us preamble + ~7.5us drain):
 - DVE: weighted products merged across iteration PAIRS ([128,2048] ops,
   still bf16 2x) to shave per-op overhead; W is packed pair-major so
   each pair's weights are contiguous. Iteration 11's products split in
   halves so the PE's final z-group h0 starts ~1.2us earlier; last two
   relu*wv fused from PSUM (scalar_tensor_tensor), iter 11 in halves.
 - PE:  z per iter: q/m0/m2/m3 identity matmuls with per-product vsem
   waits, bt4 via one-hot-d matmul from a 16KB transposed copy; final
   L-reduction via one-hot columns; L*b_v rows accumulated right after
   rr=0; warmup trimmed to 12 matmuls (p-state tolerates ~3us idle).
 - ACT: dist=abs, k=exp (slow iters only, T in bf16), relu-first
   emission, final relus straight from ps0/ps1 + output DMAs from ACT's
   DGE ring.
 - DMA: two HWDGE rings in parallel (sync: X/W/M/PD/Ident; ACT:
   T/consts/OHD/BT), ordered by first use.

Raw bass: at most ONE attached sync-wait per compute instruction, so
cross-engine deps use standalone wait_ge; each DMA gets a dedicated
semaphore (two DMAs sharing one sem can interleave per-queue
completions, so a partial wait would be unsound).
"""

import os
import numpy as np
import ml_dtypes

import concourse.bass as bass
import concourse.mybir as mybir
from concourse.bass_utils import run_bass_kernel_spmd

AF = mybir.ActivationFunctionType
OP = mybir.AluOpType
BF16 = mybir.dt.bfloat16
F32 = mybir.dt.float32

B, R, L, D = 32, 48, 128, 64
NB, NK = 2, 4              # b-halves x r-buckets = 8 cores
BC, RC = B // NB, R // NK  # 16 b, 12 r per core
FD = BC * D                # 1024 free elems
NP = RC // 2               # iteration pairs

CFC_W = 2 * RC             # f32 consts: [Abc 12 | NASbc 12]
# packed bf16 const layout: [Xt | Mt | PDt | OH 144 | I 128]
CB_W = 3 * FD + RC * RC + L
PW_W = 10 * D              # per-PAIR param slice: [w0 w0' w1 w1' w2 w2' w3 w3' wv wv']
BT_W = RC * L + 16         # bt4^T per iter + L*b_v^T columns

_nbf16 = ml_dtypes.bfloat16

LB = 5    # wl buffers (DVE -> PE)
LAB = 4   # lat buffers (ACT -> DVE)
PRBP = 2  # product buffer PAIR sets (DVE -> PE), 2 pairs = 4 iters deep
ZB = 3    # psum z triple-buffer (6 of 8 banks; ps0/ps1 take the rest)
WLAG = 3  # wl(i-WLAG) emitted around DVE iter i
ALAG = 2  # lat(i-ALAG) emitted in ACT iter i
NWARM = 12  # PE warmup matmuls (keep PE out of low p-state until work)


def _fast_mask(nfast):
    """Fast slots first, plus the last two slots when available."""
    tail = min(2, nfast)
    lead = nfast - tail
    mask = [False] * RC
    for i in range(lead):
        mask[i] = True
    for i in range(RC - tail, RC):
        mask[i] = True
    return tuple(mask)


def _dve_ops(mask):
    """DVE emission order as (key, ...) tuples; single source of truth for
    both the vsem counter table and the @block.vector body."""
    ops = [("xp", 0)]
    for j in range(NP):
        a, b = 2 * j, 2 * j + 1
        lastpair = j == NP - 1
        if a >= WLAG:
            ops.append(("wl", a - WLAG))
        if not lastpair:
            if mask[a] and mask[b]:
                ops.append(("qp", j))
            else:
                for i in (a, b):
                    if not mask[i]:
                        ops.append(("g", i))
                    ops.append(("q", i))
            ops.append(("m0p", j))
            if b >= WLAG:
                ops.append(("wl", b - WLAG))
            ops.append(("m2p", j))
            ops.append(("m3p", j))
        else:
            # iter 10 full-width; iter 11 split in halves so PE h0 starts
            # as early as possible on the drain-critical last group
            for i in (a,):
                if not mask[i]:
                    ops.append(("g", i))
                ops.append(("q", i))
            ops.append(("m0", a))
            ops.append(("wl", b - WLAG))
            ops.append(("m2", a))
            ops.append(("m3", a))
            if not mask[b]:
                ops.append(("g", b))
            for h in ("a", "b"):
                ops.append((f"q{h}", b))
                ops.append((f"m0{h}", b))
                ops.append((f"m2{h}", b))
                ops.append((f"m3{h}", b))
    ops.append(("wl", RC - 3))
    ops.append(("wlp", RC - 2))
    ops.append(("wlpa", RC - 1))
    ops.append(("wlpb", RC - 1))
    return ops


def _build_graph(mask, detect_races=True):
    nslow = sum(1 for f in mask if not f)
    ksl = max(nslow, 1)
    slows = [i for i in range(RC) if not mask[i]]
    kidx = {i: j for j, i in enumerate(slows)}  # slow iter -> k slot
    ops = _dve_ops(mask)
    C = {key: idx + 1 for idx, key in enumerate(ops)}

    def qkey(i):
        j = i // 2
        if j < NP - 1 and mask[2 * j] and mask[2 * j + 1]:
            return ("qp", j)
        return ("q", i)

    def mkey(tag, i):
        if i >= RC - 2:
            return (tag, i) if i == RC - 2 else None
        return (f"{tag}p", i // 2)

    nc = bass.Bass(detect_race_conditions=detect_races)
    cfc_e = nc.declare_dram_parameter("cfc", [L, CFC_W], F32, isOutput=False)
    cft_e = nc.declare_dram_parameter("cfT", [L, FD], BF16, isOutput=False)
    cb_e = nc.declare_dram_parameter("cb", [L, CB_W], BF16, isOutput=False)
    W_e = nc.declare_dram_parameter("W", [L, NP * PW_W], BF16, isOutput=False)
    bt_e = nc.declare_dram_parameter("BT", [64, BT_W], BF16, isOutput=False)
    oh_e = nc.declare_dram_parameter("OHD", [64, 512], BF16, isOutput=False)
    out_e = nc.declare_dram_parameter("out", [RC, FD], F32, isOutput=True)

    from contextlib import ExitStack

    with ExitStack() as ctx:
        e = ctx.enter_context
        cfc = e(nc.sbuf_tensor([L, CFC_W], F32))
        cft = e(nc.sbuf_tensor([L, FD], BF16))
        cb = e(nc.sbuf_tensor([L, CB_W], BF16))
        Wb = e(nc.sbuf_tensor([L, NP * PW_W], BF16))
        BT = e(nc.sbuf_tensor([64, BT_W], BF16))
        OHD = e(nc.sbuf_tensor([64, 512], BF16))
        Xp = e(nc.sbuf_tensor([L, FD], BF16))
        dist = e(nc.sbuf_tensor([L, FD], F32))
        kbuf = e(nc.sbuf_tensor([L, ksl * FD], BF16))
        g = e(nc.sbuf_tensor([L, FD], BF16))
        qb = e(nc.sbuf_tensor([L, PRBP * 2 * FD], BF16))
        m0b = e(nc.sbuf_tensor([L, PRBP * 2 * FD], BF16))
        m2b = e(nc.sbuf_tensor([L, PRBP * 2 * FD], BF16))
        m3b = e(nc.sbuf_tensor([L, PRBP * 2 * FD], BF16))
        latb = e(nc.sbuf_tensor([L, LAB * FD], BF16))
        wlbuf = e(nc.sbuf_tensor([L, LB * FD], BF16))
        outt = e(nc.sbuf_tensor([RC, FD], F32))
        wsc = e(nc.sbuf_tensor([L, 512], BF16))
        psz = [e(nc.psum_tensor(f"psz{j}", [L, FD], F32)) for j in range(ZB)]
        ps0 = e(nc.psum_tensor([RC, 512], F32))
        ps1 = e(nc.psum_tensor([RC, 512], F32))
        cfcsem = e(nc.semaphore("cfcsem"))
        cftsem = e(nc.semaphore("cftsem"))
        cbsem = e(nc.semaphore("cbsem"))    # cb X part
        cbmsem = e(nc.semaphore("cbmsem"))  # cb M part
        cbpsem = e(nc.semaphore("cbpsem"))  # cb PD part
        cbtsem = e(nc.semaphore("cbtsem"))  # cb OH3+Ident part
        wsem0 = e(nc.semaphore("wsem0"))    # W pair 0
        wsema = e(nc.semaphore("wsema"))    # W pairs 1-2
        wsemb = e(nc.semaphore("wsemb"))    # W pairs 3-5
        ohsem = e(nc.semaphore("ohsem"))    # OHD
        btsem = e(nc.semaphore("btsem"))    # BT
        asem = e(nc.semaphore("asem"))      # ACT k completions
        lsem = e(nc.semaphore("lsem"))      # ACT lat relu completions
        zsem = e(nc.semaphore("zsem"))      # PE z-group completions
        z2sem = e(nc.semaphore("z2sem"))    # PE last z-group half completions
        msem = e(nc.semaphore("msem"))      # PE out-mm completions
        vsem = e(nc.semaphore("vsem"))      # DVE op completions
        osem = e(nc.semaphore("osem"))
        gsem = e(nc.semaphore("gsem"))
        block = e(nc.Block())

        Abc = cfc[:, 0:RC]
        NASbc = cfc[:, RC : 2 * RC]
        Xt = cb[:, 0:FD]
        Mt = cb[:, FD : 2 * FD]
        PDt = cb[:, 2 * FD : 3 * FD]
        OH3 = cb[:, 3 * FD : 3 * FD + RC * RC].rearrange("p (r m) -> p r m", r=RC)
        Ident = cb[:, 3 * FD + RC * RC :]
        lbvT = BT[:, RC * L : RC * L + RC]

        def r3(ap):
            return ap.rearrange("p (b d) -> p b d", b=BC)

        def r3h(ap):
            return ap.rearrange("p (b d) -> p b d", b=BC // 2)

        def kslot(j):
            return kbuf[:, (j % ksl) * FD : (j % ksl + 1) * FD]

        def wsl(i, ei):
            # per-iter weight column block in the pair-major W layout
            base = (i // 2) * PW_W + ei * 2 * D + (i % 2) * D
            return Wb[:, base : base + D]

        def wbc(i, ei):
            return wsl(i, ei).unsqueeze(1).broadcast_to([L, BC, D])

        def wbch(i, ei):
            return wsl(i, ei).unsqueeze(1).broadcast_to([L, BC // 2, D])

        def wpair(j, ei):
            base = j * PW_W + ei * 2 * D
            return (
                Wb[:, base : base + 2 * D]
                .rearrange("p (t d) -> p t d", t=2)
                .unsqueeze(2)
                .broadcast_to([L, 2, BC, D])
            )

        def btsl(i):
            return BT[:, i * L : (i + 1) * L]

        def latslot(rr):
            return latb[:, (rr % LAB) * FD : (rr % LAB + 1) * FD]

        def wlslot(rr):
            return wlbuf[:, (rr % LB) * FD : (rr % LB + 1) * FD]

        def pairslot(buf, j):
            return buf[:, (j % PRBP) * 2 * FD : (j % PRBP + 1) * 2 * FD]

        def prod(buf, i):
            return pairslot(buf, i // 2)[:, (i % 2) * FD : (i % 2 + 1) * FD]

        @block.sync
        def _(sp):
            sp.dma_start(out=cb[:, 0:FD], in_=cb_e[:, 0:FD]).then_inc(cbsem, 16)
            sp.dma_start(out=Wb[:, 0:PW_W], in_=W_e[:, 0:PW_W]).then_inc(wsem0, 16)
            sp.dma_start(out=cb[:, FD : 2 * FD], in_=cb_e[:, FD : 2 * FD]).then_inc(
                cbmsem, 16
            )
            sp.dma_start(
                out=cb[:, 2 * FD : 3 * FD], in_=cb_e[:, 2 * FD : 3 * FD]
            ).then_inc(cbpsem, 16)
            sp.dma_start(
                out=Wb[:, PW_W : 3 * PW_W], in_=W_e[:, PW_W : 3 * PW_W]
            ).then_inc(wsema, 16)
            sp.dma_start(out=cb[:, 3 * FD :], in_=cb_e[:, 3 * FD :]).then_inc(
                cbtsem, 16
            )
            sp.dma_start(
                out=Wb[:, 3 * PW_W :], in_=W_e[:, 3 * PW_W :]
            ).then_inc(wsemb, 16)

        @block.scalar
        def _(act):
            act.dma_start(out=cfc[:, :], in_=cfc_e[:, :]).then_inc(cfcsem, 16)
            act.dma_start(out=cft[:, :], in_=cft_e[:, :]).then_inc(cftsem, 16)
            act.dma_start(out=OHD[:, :], in_=oh_e[:, :]).then_inc(ohsem, 16)
            act.dma_start(out=BT[:, :], in_=bt_e[:, :]).then_inc(btsem, 16)
            act.wait_ge(cfcsem, 16)
            act.wait_ge(cftsem, 16)
            for i in range(RC):
                # relus rr=0..9 only; last two z's go through the DVE wlp path
                if ALAG <= i and i - ALAG <= RC - WLAG:
                    rr = i - ALAG
                    act.wait_ge(zsem, rr + 1)
                    if rr >= LAB:
                        act.wait_ge(vsem, C[("wl", rr - LAB)])
                    nc.scalar.activation(
                        latslot(rr), psz[rr % ZB][:, :], AF.Relu
                    ).then_inc(lsem, 1)
                if i < nslow:
                    si = slows[i]
                    nc.scalar.activation(
                        dist[:, :], cft[:, :], AF.Abs,
                        bias=NASbc[:, si : si + 1], scale=Abc[:, si : si + 1],
                    )
                    nc.scalar.activation(
                        kslot(i), dist[:, :], AF.Exp, scale=-1.0
                    ).then_inc(asem, 1)
            act.wait_ge(msem, 12)
            nc.scalar.activation(outt[:, 0:512], ps0[:, :], AF.Relu)
            act.dma_start(out=out_e[:, 0:512], in_=outt[:, 0:512]).then_inc(osem, 16)
            act.wait_ge(msem, 13)
            nc.scalar.activation(outt[:, 512:], ps1[:, :], AF.Relu)
            act.dma_start(out=out_e[:, 512:], in_=outt[:, 512:]).then_inc(osem, 16)

        @block.vector
        def _(ve):
            ve.wait_ge(cbsem, 16)
            nc.vector.tensor_scalar_max(Xp[:, :], Xt, 0.0).then_inc(vsem, 1)
            emitted_m = set()
            for key in _dve_ops(mask)[1:]:
                tag = key[0]
                if tag == "wl":
                    rr = key[1]
                    ve.wait_ge(lsem, rr + 1)
                    if rr >= LB:
                        ve.wait_ge(msem, rr - LB + 1)
                    nc.vector.tensor_tensor(
                        r3(wlslot(rr)), r3(latslot(rr)), wbc(rr, 4), OP.mult
                    ).then_inc(vsem, 1)
                elif tag == "wlp":
                    rr = key[1]
                    ve.wait_ge(zsem, rr + 1)
                    ve.wait_ge(msem, rr - LB + 1)
                    nc.vector.scalar_tensor_tensor(
                        r3(wlslot(rr)), r3(psz[rr % ZB][:, :]), 0.0, wbc(rr, 4),
                        OP.max, OP.mult,
                    ).then_inc(vsem, 1)
                elif tag in ("wlpa", "wlpb"):
                    rr = key[1]
                    h = 0 if tag == "wlpa" else 1
                    if h == 0:
                        ve.wait_ge(msem, rr - LB + 1)
                    ve.wait_ge(z2sem, h + 1)
                    c0, c1 = h * 512, (h + 1) * 512
                    nc.vector.scalar_tensor_tensor(
                        r3h(wlslot(rr)[:, c0:c1]),
                        r3h(psz[rr % ZB][:, c0:c1]),
                        0.0, wbch(rr, 4), OP.max, OP.mult,
                    ).then_inc(vsem, 1)
                elif tag == "g":
                    i = key[1]
                    ve.wait_ge(asem, kidx[i] + 1)
                    nc.vector.tensor_mul(
                        g[:, :], Xp[:, :], kslot(kidx[i])
                    ).then_inc(vsem, 1)
                elif tag in ("q", "qp", "qa", "qb"):
                    i = key[1]
                    j = i if tag == "qp" else i // 2
                    if j == 0 and not emitted_m:
                        ve.wait_ge(wsem0, 16)
                    elif j == 1 and "w1" not in emitted_m:
                        ve.wait_ge(wsema, 16)
                        emitted_m.add("w1")
                    elif j == 3 and "w3" not in emitted_m:
                        ve.wait_ge(wsemb, 16)
                        emitted_m.add("w3")
                    emitted_m.add("w0")
                    if tag == "qp":
                        nc.vector.tensor_tensor(
                            pairslot(qb, j).rearrange(
                                "p (t b d) -> p t b d", t=2, b=BC
                            ),
                            r3(Xp[:, :]).unsqueeze(1).broadcast_to([L, 2, BC, D]),
                            wpair(j, 1),
                            OP.mult,
                        ).then_inc(vsem, 1)
                    elif tag == "q":
                        qsrc = g[:, :] if not mask[i] else Xp[:, :]
                        nc.vector.tensor_tensor(
                            r3(prod(qb, i)), r3(qsrc), wbc(i, 1), OP.mult
                        ).then_inc(vsem, 1)
                    else:
                        h = 0 if tag == "qa" else 1
                        c0, c1 = h * 512, (h + 1) * 512
                        qsrc = g[:, :] if not mask[i] else Xp[:, :]
                        nc.vector.tensor_tensor(
                            r3h(prod(qb, i)[:, c0:c1]),
                            r3h(qsrc[:, c0:c1]),
                            wbch(i, 1), OP.mult,
                        ).then_inc(vsem, 1)
                else:
                    # m-products: m0/m2/m3 in pair, single, or half variants
                    mt = {"m0": (m0b, 0, Xt), "m2": (m2b, 2, Mt), "m3": (m3b, 3, PDt)}
                    base = tag[:2]
                    buf, ei, src = mt[base]
                    if base == "m2" and "m2w" not in emitted_m:
                        ve.wait_ge(cbmsem, 16)
                        emitted_m.add("m2w")
                    if base == "m3" and "m3w" not in emitted_m:
                        ve.wait_ge(cbpsem, 16)
                        emitted_m.add("m3w")
                    if tag.endswith("p"):
                        j = key[1]
                        nc.vector.tensor_tensor(
                            pairslot(buf, j).rearrange(
                                "p (t b d) -> p t b d", t=2, b=BC
                            ),
                            r3(src).unsqueeze(1).broadcast_to([L, 2, BC, D]),
                            wpair(j, ei),
                            OP.mult,
                        ).then_inc(vsem, 1)
                    elif tag.endswith("a") or tag.endswith("b"):
                        i = key[1]
                        h = 0 if tag.endswith("a") else 1
                        c0, c1 = h * 512, (h + 1) * 512
                        nc.vector.tensor_tensor(
                            r3h(prod(buf, i)[:, c0:c1]),
                            r3h(src[:, c0:c1]),
                            wbch(i, ei), OP.mult,
                        ).then_inc(vsem, 1)
                    else:
                        i = key[1]
                        nc.vector.tensor_tensor(
                            r3(prod(buf, i)), r3(src), wbc(i, ei), OP.mult
                        ).then_inc(vsem, 1)

        @block.gpsimd
        def _(gp):
            nc.gpsimd.memset(wsc[:, :], 1.0).then_inc(gsem, 1)

        @block.tensor
        def _(te):
            # warmup: keep the PE out of its low p-state until real work
            # arrives. Results never read; ps0 reset by the real start=True.
            te.wait_ge(gsem, 1)
            for _w in range(NWARM):
                nc.tensor.matmul(
                    ps0[:, :], wsc[:, 0:RC], wsc[:, :],
                    start=True, stop=True, skip_group_check=True,
                )
            te.wait_ge(ohsem, 16)
            te.wait_ge(btsem, 16)
            te.wait_ge(cbtsem, 16)
            for i in range(RC):
                last = i == RC - 1
                if i >= ZB:
                    te.wait_ge(lsem, i - ZB + 1)
                pz = psz[i % ZB]
                prods = ((qb, "q"), (m0b, "m0"), (m2b, "m2"), (m3b, "m3"))
                if last:
                    # h0 stream first, then h1, each closed separately so the
                    # DVE's wlp halves overlap with this group's tail
                    for h, hs in ((0, "a"), (1, "b")):
                        for pb, tg in prods:
                            k = qkey(i) if tg == "q" else mkey(tg, i)
                            k = (f"{tg}{hs}", i)
                            te.wait_ge(vsem, C[k])
                            c0, c1 = h * 512, (h + 1) * 512
                            nc.tensor.matmul(
                                pz[:, c0:c1], Ident, prod(pb, i)[:, c0:c1],
                                start=(tg == "q"), stop=False,
                                skip_group_check=True,
                            )
                        nc.tensor.matmul(
                            pz[:, h * 512 : (h + 1) * 512], btsl(i), OHD[:, :],
                            start=False, stop=True, skip_group_check=True,
                        ).then_inc(z2sem, 1)
                else:
                    for pb, tg in prods:
                        k = qkey(i) if tg == "q" else mkey(tg, i)
                        te.wait_ge(vsem, C[k])
                        for h in (0, 1):
                            c0, c1 = h * 512, (h + 1) * 512
                            nc.tensor.matmul(
                                pz[:, c0:c1], Ident, prod(pb, i)[:, c0:c1],
                                start=(tg == "q"), stop=False,
                                skip_group_check=True,
                            )
                    for h in (0, 1):
                        mm = nc.tensor.matmul(
                            pz[:, h * 512 : (h + 1) * 512], btsl(i), OHD[:, :],
                            start=False, stop=True, skip_group_check=True,
                        )
                        if h == 1:
                            mm.then_inc(zsem, 1)
                if i >= WLAG:
                    rr = i - WLAG
                    te.wait_ge(vsem, C[("wl", rr)])
                    wl = wlslot(rr)
                    nc.tensor.matmul(
                        ps0[:, :], OH3[:, rr, :], wl[:, 0:512],
                        start=(rr == 0), stop=False, skip_group_check=True,
                    )
                    nc.tensor.matmul(
                        ps1[:, :], OH3[:, rr, :], wl[:, 512:1024],
                        start=(rr == 0), stop=False, skip_group_check=True,
                    ).then_inc(msem, 1)
                    if rr == 0:
                        # accumulate the L*b_v rows early (order irrelevant)
                        nc.tensor.matmul(
                            ps0[:, :], lbvT, OHD[:, 0:512],
                            start=False, stop=False, skip_group_check=True,
                        )
                        nc.tensor.matmul(
                            ps1[:, :], lbvT, OHD[:, 0:512],
                            start=False, stop=False, skip_group_check=True,
                        )
            for rr in (RC - 3, RC - 2):
                key = ("wl", rr) if rr < RC - 2 else ("wlp", rr)
                te.wait_ge(vsem, C[key])
                wl = wlslot(rr)
                nc.tensor.matmul(
                    ps0[:, :], OH3[:, rr, :], wl[:, 0:512],
                    start=False, stop=False, skip_group_check=True,
                )
                nc.tensor.matmul(
                    ps1[:, :], OH3[:, rr, :], wl[:, 512:1024],
                    start=False, stop=False, skip_group_check=True,
                ).then_inc(msem, 1)
            rr = RC - 1
            wl = wlslot(rr)
            te.wait_ge(vsem, C[("wlpa", rr)])
            nc.tensor.matmul(
                ps0[:, :], OH3[:, rr, :], wl[:, 0:512],
                start=False, stop=True, skip_group_check=True,
            ).then_inc(msem, 1)
            te.wait_ge(vsem, C[("wlpb", rr)])
            nc.tensor.matmul(
                ps1[:, :], OH3[:, rr, :], wl[:, 512:1024],
                start=False, stop=True, skip_group_check=True,
            ).then_inc(msem, 1)

    return nc


_CACHE = {}


def _buckets(a):
    """Deal r-indices into NK buckets of RC; zeros occupy each bucket's
    fast-mask positions first. Returns (buckets, nfast)."""
    zeros = [r for r in range(R) if a[r] == 0.0]
    pos = [r for r in range(R) if a[r] != 0.0]
    zbuck = [[] for _ in range(NK)]
    for j, r in enumerate(zeros):
        zbuck[j % NK].append(r)
    nfast = min(min(len(zb) for zb in zbuck), RC)
    mask = _fast_mask(nfast)
    pi = 0
    buckets = []
    for k in range(NK):
        zq = list(zbuck[k])
        rl = [None] * RC
        for i in range(RC):
            if mask[i]:
                rl[i] = zq.pop(0)
        for i in range(RC):
            if rl[i] is None:
                if zq:
                    rl[i] = zq.pop(0)
                else:
                    rl[i] = pos[pi]
                    pi += 1
        buckets.append(rl)
    return buckets, nfast


def _prepare(X, T, M, PD, alpha, w_v, w_t, b_t, b_v, ref_time):
    """Pack full inputs into per-core DRAM parameter maps.
    Returns (mask, buckets, in_maps)."""
    a = np.maximum(alpha.reshape(R), 0.0)
    s_ref = ref_time.reshape(R)
    nas = -(a * s_ref)
    bt4 = 4.0 * b_t[..., 0]              # [R, L, D]
    lbv = float(L) * b_v[:, 0, :]        # [R, D]

    buckets, nfast = _buckets(a)
    mask = _fast_mask(nfast)

    # pair-major params: per pair j the 10 blocks [w0 w0' w1 w1' ... wv wv']
    wts = np.stack(
        [w_t[..., 0], w_t[..., 1], w_t[..., 2], w_t[..., 3], w_v], axis=2
    )                                     # [R, L, 5, D]

    oh = np.zeros((L, RC, RC), np.float32)
    for r in range(RC):
        oh[:, r, r] = 1.0
    ident = np.eye(L, dtype=np.float32)
    ohd = np.zeros((64, 512), np.float32)
    for b in range(8):
        for d in range(64):
            ohd[d, b * 64 + d] = 1.0

    in_maps = []
    for c in range(8):
        b0 = (c // NK) * BC
        rl = buckets[c % NK]
        tr = lambda x: np.ascontiguousarray(
            x[b0 : b0 + BC].transpose(1, 0, 2).reshape(L, FD)
        )
        cfc = np.zeros((L, CFC_W), np.float32)
        cfc[:, 0:RC] = a[rl]
        cfc[:, RC : 2 * RC] = nas[rl]
        cbf = np.zeros((L, CB_W), np.float32)
        cbf[:, 0:FD] = tr(X)
        cbf[:, FD : 2 * FD] = tr(M)
        cbf[:, 2 * FD : 3 * FD] = tr(PD)
        cbf[:, 3 * FD : 3 * FD + RC * RC] = oh.reshape(L, RC * RC)
        cbf[:, 3 * FD + RC * RC :] = ident
        wp = np.zeros((L, NP * PW_W), np.float32)
        for i, r in enumerate(rl):
            j, t = i // 2, i % 2
            for ei in range(5):
                col = j * PW_W + ei * 2 * D + t * D
                wp[:, col : col + D] = wts[r, :, ei, :]
        btp = np.zeros((64, BT_W), np.float32)
        for i, r in enumerate(rl):
            btp[0:D, i * L : (i + 1) * L] = bt4[r].T
            btp[0:D, RC * L + i] = lbv[r]
        in_maps.append(
            {
                "cfc": cfc,
                "cfT": tr(T).astype(_nbf16),
                "cb": cbf.astype(_nbf16),
                "W": wp.astype(_nbf16),
                "BT": btp.astype(_nbf16),
                "OHD": ohd.astype(_nbf16),
            }
        )
    return mask, buckets, in_maps


def kernel(X, T, M, PD, alpha, w_v, w_t, b_t, b_v, ref_time):
    X = np.asarray(X, np.float32)
    T = np.asarray(T, np.float32)
    M = np.asarray(M, np.float32)
    PD = np.asarray(PD, np.float32)
    alpha = np.asarray(alpha, np.float32)
    w_v = np.asarray(w_v, np.float32)
    w_t = np.asarray(w_t, np.float32)
    b_t = np.asarray(b_t, np.float32)
    b_v = np.asarray(b_v, np.float32)
    ref_time = np.asarray(ref_time, np.float32)

    mask, buckets, in_maps = _prepare(
        X, T, M, PD, alpha, w_v, w_t, b_t, b_v, ref_time
    )

    if mask not in _CACHE:
        _CACHE[mask] = _build_graph(mask)
    nc = _CACHE[mask]

    trace = bool(os.environ.get("BASS_KERNEL_TRACE"))
    kw = {}
    if trace:
        tmpdir = os.environ.get("BASS_KERNEL_TRACE_DIR") or None
        kw = dict(trace=True, tmpdir=tmpdir)
    res = run_bass_kernel_spmd(nc, in_maps, core_ids=list(range(8)), **kw)
    if trace:
        _CACHE["exec_time_ns"] = res.exec_time_ns
        print(f"HW exec time: {res.exec_time_ns} ns")

    out = np.zeros((B, R, D), np.float32)
    for c in range(8):
        b0 = (c // NK) * BC
        rl = buckets[c % NK]
        o = np.asarray(res.results[c]["out"], np.float32).reshape(RC, BC, D)
        for i, r in enumerate(rl):
            out[b0 : b0 + BC, r] = o[i]
    return out


# revision 14
# speedup vs baseline: 1.0281x; 1.0281x over previous
"""ALNN layer kernel for 8 TRN2 NeuronCores (raw Bass, explicit semaphores).

out[b,r,d] = relu( sum_l w_v[r,l,d]*relu(z[b,r,l,d]) + L*b_v[r,d] )
z = wt0*X + wt1*relu(X)*k + wt2*M + wt3*PD + 4*bt
k = exp(-relu(alpha_r)*|T - s_r|)        (uses relu(X*k) == relu(X)*k, k>0)

Sharding: B split 2 ways x R dealt into 4 buckets -> 8 cores, 16 b x 12 r
each. Cores c and c+4 share bucket c%4. r's with relu(alpha)=0 take a
fast path (k == 1: ABS/EXP/g skipped, q = relu(X)*wt1). The graph is
SPMD-uniform: a per-iteration fast-mask shared by all cores, sized by the
min zero-count across buckets (extra zeros run the slow path, still
correct since exp(-0*dist) == 1). Fast slots sit at the start (early DVE
start without waiting for ACT's k) and the last two slots (shorter drain).

Per-core layout: partitions = L(128), free = (b=16, d=64) = 1024.

v7 engine plan (v5 measured 59563ns; v6's iteration-pair merging REGRESSED
to 63247 — pair products delay the fill-phase z-groups for a negligible
busy saving — so v7 is v5's per-iteration structure with the early-DMA
contention fix; fixed ~6.5us preamble + ~7.5us drain bracket everything):
 - DVE: 4-5 weighted products per iter + wl (bf16 2x mode). Last two
   iterations' relu*wv fused as scalar_tensor_tensor from PSUM, the very
   last split in halves to pipeline the drain.
 - PE:  z in PSUM per iter: q/m0/m2/m3 identity matmuls (per-product
   vsem waits so the group starts as soon as q lands), then bt4
   reconstructed from a transposed 16KB copy via one-hot-d matmul;
   final L-reduction via one-hot columns; L*b_v rows accumulated right
   after rr=0.
 - ACT: dist=abs, k=exp (slow iters only, T in bf16), lat=relu(PSUM z)
   emitted relu-first each iteration, final relus straight from ps0/ps1,
   and the two output-half DMAs issued from ACT's own DGE ring.
 - DMA: ACT's ring carries only the 280KB cfc+cfT (v5 also put OHD/BT
   there, and the 1:1 queue interleave starved the sync stream: m2(0)
   stalled 2.1us on Mt). Sync ring order matches first use:
   X, W0, M, W1, PD, Ident/OH, OHD, BT, W2-5, W6-11.

Raw bass: at most ONE attached sync-wait per compute instruction, so
cross-engine deps use standalone wait_ge; each DMA gets a dedicated
semaphore (two DMAs sharing one sem can interleave per-queue completions,
so a partial wait would be unsound).
"""

import os
import numpy as np
import ml_dtypes

import concourse.bass as bass
import concourse.mybir as mybir
from concourse.bass_utils import run_bass_kernel_spmd

AF = mybir.ActivationFunctionType
OP = mybir.AluOpType
BF16 = mybir.dt.bfloat16
F32 = mybir.dt.float32

B, R, L, D = 32, 48, 128, 64
NB, NK = 2, 4              # b-halves x r-buckets = 8 cores
BC, RC = B // NB, R // NK  # 16 b, 12 r per core
FD = BC * D                # 1024 free elems

CFC_W = 2 * RC             # f32 consts: [Abc 12 | NASbc 12]
# packed bf16 const layout: [Xt | Mt | PDt | OH 144 | I 128]
CB_W = 3 * FD + RC * RC + L
WS_W = 5 * D               # per-iter param slice: [wt0|wt1|wt2|wt3|wv]
BT_W = RC * L + 16         # bt4^T per iter + L*b_v^T columns

_nbf16 = ml_dtypes.bfloat16

LB = 5   # wl buffers (DVE -> PE)
LAB = 4  # lat buffers (ACT -> DVE)
PRB = 3  # product buffer sets (DVE -> PE)
ZB = 3   # psum z triple-buffer (6 of 8 banks; ps0/ps1 take the rest)
WLAG = 3  # wl(i-WLAG) emitted in DVE iter i
ALAG = 2  # lat(i-ALAG) emitted in ACT iter i
NWARM = 24  # PE warmup matmuls (keep PE out of low p-state until work)


def _fast_mask(nfast):
    """Fast slots first, plus the last two slots when available."""
    tail = min(2, nfast)
    lead = nfast - tail
    mask = [False] * RC
    for i in range(lead):
        mask[i] = True
    for i in range(RC - tail, RC):
        mask[i] = True
    return tuple(mask)


def _dve_schedule(mask):
    """DVE emission order; must match the @block.vector body exactly."""
    sched = [("xp", 0)]
    for i in range(RC):
        if i >= WLAG:
            sched.append(("wl", i - WLAG))
        if not mask[i]:
            sched.append(("g", i))
        sched.append(("q", i))
        sched.append(("m0", i))
        sched.append(("m2", i))
        sched.append(("m3", i))
    for rr in range(RC - WLAG, RC - 2):
        sched.append(("wl", rr))
    sched.append(("wlp", RC - 2))
    sched.append(("wlpa", RC - 1))
    sched.append(("wlpb", RC - 1))
    return sched


def _build_graph(mask, detect_races=True):
    nslow = sum(1 for f in mask if not f)
    ksl = max(nslow, 1)
    slows = [i for i in range(RC) if not mask[i]]
    kidx = {i: j for j, i in enumerate(slows)}  # slow iter -> k slot
    sched = _dve_schedule(mask)
    C = {key: idx + 1 for idx, key in enumerate(sched)}

    nc = bass.Bass(detect_race_conditions=detect_races)
    cfc_e = nc.declare_dram_parameter("cfc", [L, CFC_W], F32, isOutput=False)
    cft_e = nc.declare_dram_parameter("cfT", [L, FD], BF16, isOutput=False)
    cb_e = nc.declare_dram_parameter("cb", [L, CB_W], BF16, isOutput=False)
    W_e = nc.declare_dram_parameter("W", [L, RC * WS_W], BF16, isOutput=False)
    bt_e = nc.declare_dram_parameter("BT", [64, BT_W], BF16, isOutput=False)
    oh_e = nc.declare_dram_parameter("OHD", [64, 512], BF16, isOutput=False)
    out_e = nc.declare_dram_parameter("out", [RC, FD], F32, isOutput=True)

    from contextlib import ExitStack

    with ExitStack() as ctx:
        e = ctx.enter_context
        cfc = e(nc.sbuf_tensor([L, CFC_W], F32))
        cft = e(nc.sbuf_tensor([L, FD], BF16))
        cb = e(nc.sbuf_tensor([L, CB_W], BF16))
        Wb = e(nc.sbuf_tensor([L, RC * WS_W], BF16))
        BT = e(nc.sbuf_tensor([64, BT_W], BF16))
        OHD = e(nc.sbuf_tensor([64, 512], BF16))
        Xp = e(nc.sbuf_tensor([L, FD], BF16))
        dist = e(nc.sbuf_tensor([L, FD], F32))
        kbuf = e(nc.sbuf_tensor([L, ksl * FD], BF16))
        g = e(nc.sbuf_tensor([L, FD], BF16))
        qb = e(nc.sbuf_tensor([L, PRB * FD], BF16))
        m0b = e(nc.sbuf_tensor([L, PRB * FD], BF16))
        m2b = e(nc.sbuf_tensor([L, PRB * FD], BF16))
        m3b = e(nc.sbuf_tensor([L, PRB * FD], BF16))
        latb = e(nc.sbuf_tensor([L, LAB * FD], BF16))
        wlbuf = e(nc.sbuf_tensor([L, LB * FD], BF16))
        outt = e(nc.sbuf_tensor([RC, FD], F32))
        wsc = e(nc.sbuf_tensor([L, 512], BF16))
        psz = [e(nc.psum_tensor(f"psz{j}", [L, FD], F32)) for j in range(ZB)]
        ps0 = e(nc.psum_tensor([RC, 512], F32))
        ps1 = e(nc.psum_tensor([RC, 512], F32))
        cfcsem = e(nc.semaphore("cfcsem"))
        cftsem = e(nc.semaphore("cftsem"))
        cbsem = e(nc.semaphore("cbsem"))    # cb X part
        cbmsem = e(nc.semaphore("cbmsem"))  # cb M part
        cbpsem = e(nc.semaphore("cbpsem"))  # cb PD part
        cbtsem = e(nc.semaphore("cbtsem"))  # cb OH3+Ident part
        wsem0 = e(nc.semaphore("wsem0"))    # W iter 0
        wsem1 = e(nc.semaphore("wsem1"))    # W iter 1
        wsema = e(nc.semaphore("wsema"))    # W iters 2-5
        wsemb = e(nc.semaphore("wsemb"))    # W iters 6-11
        ohsem = e(nc.semaphore("ohsem"))    # OHD
        btsem = e(nc.semaphore("btsem"))    # BT
        asem = e(nc.semaphore("asem"))      # ACT k completions
        lsem = e(nc.semaphore("lsem"))      # ACT lat relu completions
        zsem = e(nc.semaphore("zsem"))      # PE z-group completions (1/iter)
        z2sem = e(nc.semaphore("z2sem"))    # PE last z-group half completions
        msem = e(nc.semaphore("msem"))      # PE out-mm completions
        vsem = e(nc.semaphore("vsem"))      # DVE op completions
        osem = e(nc.semaphore("osem"))
        gsem = e(nc.semaphore("gsem"))
        block = e(nc.Block())

        Abc = cfc[:, 0:RC]
        NASbc = cfc[:, RC : 2 * RC]
        Xt = cb[:, 0:FD]
        Mt = cb[:, FD : 2 * FD]
        PDt = cb[:, 2 * FD : 3 * FD]
        OH3 = cb[:, 3 * FD : 3 * FD + RC * RC].rearrange("p (r m) -> p r m", r=RC)
        Ident = cb[:, 3 * FD + RC * RC :]
        lbvT = BT[:, RC * L : RC * L + RC]

        def r3(ap):
            return ap.rearrange("p (b d) -> p b d", b=BC)

        def kslot(j):
            return kbuf[:, (j % ksl) * FD : (j % ksl + 1) * FD]

        def wbc(i, ei):
            base = i * WS_W + ei * D
            return Wb[:, base : base + D].unsqueeze(1).broadcast_to([L, BC, D])

        def wbch(i, ei):
            base = i * WS_W + ei * D
            return (
                Wb[:, base : base + D].unsqueeze(1).broadcast_to([L, BC // 2, D])
            )

        def btsl(i):
            return BT[:, i * L : (i + 1) * L]

        def latslot(rr):
            return latb[:, (rr % LAB) * FD : (rr % LAB + 1) * FD]

        def wlslot(rr):
            return wlbuf[:, (rr % LB) * FD : (rr % LB + 1) * FD]

        def prod(buf, i):
            return buf[:, (i % PRB) * FD : (i % PRB + 1) * FD]

        @block.sync
        def _(sp):
            sp.dma_start(out=cb[:, 0:FD], in_=cb_e[:, 0:FD]).then_inc(cbsem, 16)
            sp.dma_start(out=Wb[:, 0:WS_W], in_=W_e[:, 0:WS_W]).then_inc(wsem0, 16)
            sp.dma_start(out=cb[:, FD : 2 * FD], in_=cb_e[:, FD : 2 * FD]).then_inc(
                cbmsem, 16
            )
            sp.dma_start(
                out=Wb[:, WS_W : 2 * WS_W], in_=W_e[:, WS_W : 2 * WS_W]
            ).then_inc(wsem1, 16)
            sp.dma_start(
                out=cb[:, 2 * FD : 3 * FD], in_=cb_e[:, 2 * FD : 3 * FD]
            ).then_inc(cbpsem, 16)
            sp.dma_start(out=cb[:, 3 * FD :], in_=cb_e[:, 3 * FD :]).then_inc(
                cbtsem, 16
            )
            sp.dma_start(out=OHD[:, :], in_=oh_e[:, :]).then_inc(ohsem, 16)
            sp.dma_start(out=BT[:, :], in_=bt_e[:, :]).then_inc(btsem, 16)
            sp.dma_start(
                out=Wb[:, 2 * WS_W : 6 * WS_W], in_=W_e[:, 2 * WS_W : 6 * WS_W]
            ).then_inc(wsema, 16)
            sp.dma_start(
                out=Wb[:, 6 * WS_W :], in_=W_e[:, 6 * WS_W :]
            ).then_inc(wsemb, 16)

        @block.scalar
        def _(act):
            act.dma_start(out=cfc[:, :], in_=cfc_e[:, :]).then_inc(cfcsem, 16)
            act.dma_start(out=cft[:, :], in_=cft_e[:, :]).then_inc(cftsem, 16)
            act.wait_ge(cfcsem, 16)
            act.wait_ge(cftsem, 16)
            for i in range(RC):
                # relus rr=0..9 only; last two z's go through the DVE wlp path
                if ALAG <= i and i - ALAG <= RC - WLAG:
                    rr = i - ALAG
                    act.wait_ge(zsem, rr + 1)
                    if rr >= LAB:
                        act.wait_ge(vsem, C[("wl", rr - LAB)])
                    nc.scalar.activation(
                        latslot(rr), psz[rr % ZB][:, :], AF.Relu
                    ).then_inc(lsem, 1)
                if i < nslow:
                    si = slows[i]
                    nc.scalar.activation(
                        dist[:, :], cft[:, :], AF.Abs,
                        bias=NASbc[:, si : si + 1], scale=Abc[:, si : si + 1],
                    )
                    nc.scalar.activation(
                        kslot(i), dist[:, :], AF.Exp, scale=-1.0
                    ).then_inc(asem, 1)
            act.wait_ge(msem, 12)
            nc.scalar.activation(outt[:, 0:512], ps0[:, :], AF.Relu)
            act.dma_start(out=out_e[:, 0:512], in_=outt[:, 0:512]).then_inc(osem, 16)
            act.wait_ge(msem, 13)
            nc.scalar.activation(outt[:, 512:], ps1[:, :], AF.Relu)
            act.dma_start(out=out_e[:, 512:], in_=outt[:, 512:]).then_inc(osem, 16)

        @block.vector
        def _(ve):
            ve.wait_ge(cbsem, 16)
            nc.vector.tensor_scalar_max(Xp[:, :], Xt, 0.0).then_inc(vsem, 1)
            for i in range(RC):
                if i >= WLAG:
                    rr = i - WLAG
                    ve.wait_ge(lsem, rr + 1)
                    if rr >= LB:
                        ve.wait_ge(msem, rr - LB + 1)
                    nc.vector.tensor_tensor(
                        r3(wlslot(rr)), r3(latslot(rr)), wbc(rr, 4), OP.mult
                    ).then_inc(vsem, 1)
                if not mask[i]:
                    ve.wait_ge(asem, kidx[i] + 1)
                    nc.vector.tensor_mul(g[:, :], Xp[:, :], kslot(kidx[i])).then_inc(
                        vsem, 1
                    )
                    qsrc = g[:, :]
                else:
                    qsrc = Xp[:, :]
                if i == 0:
                    ve.wait_ge(wsem0, 16)
                elif i == 1:
                    ve.wait_ge(wsem1, 16)
                elif i == 2:
                    ve.wait_ge(wsema, 16)
                elif i == 6:
                    ve.wait_ge(wsemb, 16)
                nc.vector.tensor_tensor(
                    r3(prod(qb, i)), r3(qsrc), wbc(i, 1), OP.mult
                ).then_inc(vsem, 1)
                nc.vector.tensor_tensor(
                    r3(prod(m0b, i)), r3(Xt), wbc(i, 0), OP.mult
                ).then_inc(vsem, 1)
                if i == 0:
                    ve.wait_ge(cbmsem, 16)
                nc.vector.tensor_tensor(
                    r3(prod(m2b, i)), r3(Mt), wbc(i, 2), OP.mult
                ).then_inc(vsem, 1)
                if i == 0:
                    ve.wait_ge(cbpsem, 16)
                nc.vector.tensor_tensor(
                    r3(prod(m3b, i)), r3(PDt), wbc(i, 3), OP.mult
                ).then_inc(vsem, 1)
            for rr in range(RC - WLAG, RC - 2):
                ve.wait_ge(lsem, rr + 1)
                ve.wait_ge(msem, rr - LB + 1)
                nc.vector.tensor_tensor(
                    r3(wlslot(rr)), r3(latslot(rr)), wbc(rr, 4), OP.mult
                ).then_inc(vsem, 1)
            # fused relu*wv straight from PSUM for the last two iterations;
            # the very last one in halves so the out matmuls/relus pipeline
            rr = RC - 2
            ve.wait_ge(zsem, rr + 1)
            ve.wait_ge(msem, rr - LB + 1)
            nc.vector.scalar_tensor_tensor(
                r3(wlslot(rr)), r3(psz[rr % ZB][:, :]), 0.0, wbc(rr, 4),
                OP.max, OP.mult,
            ).then_inc(vsem, 1)
            rr = RC - 1
            ve.wait_ge(msem, rr - LB + 1)
            for h, zwait in ((0, 1), (1, 2)):
                ve.wait_ge(z2sem, zwait)
                c0, c1 = h * 512, (h + 1) * 512
                wl3 = wlslot(rr)[:, c0:c1].rearrange("p (b d) -> p b d", b=BC // 2)
                pz3 = psz[rr % ZB][:, c0:c1].rearrange(
                    "p (b d) -> p b d", b=BC // 2
                )
                nc.vector.scalar_tensor_tensor(
                    wl3, pz3, 0.0, wbch(rr, 4), OP.max, OP.mult
                ).then_inc(vsem, 1)

        @block.gpsimd
        def _(gp):
            nc.gpsimd.memset(wsc[:, :], 1.0).then_inc(gsem, 1)

        @block.tensor
        def _(te):
            # warmup: keep the PE out of its low p-state until real work
            # arrives. Results never read; ps0 reset by the real start=True.
            te.wait_ge(gsem, 1)
            for _w in range(NWARM):
                nc.tensor.matmul(
                    ps0[:, :], wsc[:, 0:RC], wsc[:, :],
                    start=True, stop=True, skip_group_check=True,
                )
            te.wait_ge(ohsem, 16)
            te.wait_ge(btsem, 16)
            te.wait_ge(cbtsem, 16)
            for i in range(RC):
                last = i == RC - 1
                if i >= ZB:
                    te.wait_ge(lsem, i - ZB + 1)
                pz = psz[i % ZB]
                prods = ((qb, "q"), (m0b, "m0"), (m2b, "m2"), (m3b, "m3"))
                if last:
                    # h0 stream first, then h1, each closed separately so the
                    # DVE's wlp halves overlap with this group's tail
                    for h in (0, 1):
                        for pb, tag in prods:
                            if h == 0:
                                te.wait_ge(vsem, C[(tag, i)])
                            c0, c1 = h * 512, (h + 1) * 512
                            nc.tensor.matmul(
                                pz[:, c0:c1], Ident, prod(pb, i)[:, c0:c1],
                                start=(tag == "q"), stop=False,
                                skip_group_check=True,
                            )
                        nc.tensor.matmul(
                            pz[:, h * 512 : (h + 1) * 512], btsl(i), OHD[:, :],
                            start=False, stop=True, skip_group_check=True,
                        ).then_inc(z2sem, 1)
                else:
                    for pb, tag in prods:
                        te.wait_ge(vsem, C[(tag, i)])
                        for h in (0, 1):
                            c0, c1 = h * 512, (h + 1) * 512
                            nc.tensor.matmul(
                                pz[:, c0:c1], Ident, prod(pb, i)[:, c0:c1],
                                start=(tag == "q"), stop=False,
                                skip_group_check=True,
                            )
                    for h in (0, 1):
                        mm = nc.tensor.matmul(
                            pz[:, h * 512 : (h + 1) * 512], btsl(i), OHD[:, :],
                            start=False, stop=True, skip_group_check=True,
                        )
                        if h == 1:
                            mm.then_inc(zsem, 1)
                if i >= WLAG:
                    rr = i - WLAG
                    te.wait_ge(vsem, C[("wl", rr)])
                    wl = wlslot(rr)
                    nc.tensor.matmul(
                        ps0[:, :], OH3[:, rr, :], wl[:, 0:512],
                        start=(rr == 0), stop=False, skip_group_check=True,
                    )
                    nc.tensor.matmul(
                        ps1[:, :], OH3[:, rr, :], wl[:, 512:1024],
                        start=(rr == 0), stop=False, skip_group_check=True,
                    ).then_inc(msem, 1)
                    if rr == 0:
                        # accumulate the L*b_v rows early (order irrelevant)
                        nc.tensor.matmul(
                            ps0[:, :], lbvT, OHD[:, 0:512],
                            start=False, stop=False, skip_group_check=True,
                        )
                        nc.tensor.matmul(
                            ps1[:, :], lbvT, OHD[:, 0:512],
                            start=False, stop=False, skip_group_check=True,
                        )
            for rr in range(RC - WLAG, RC - 1):
                key = ("wl", rr) if rr < RC - 2 else ("wlp", rr)
                te.wait_ge(vsem, C[key])
                wl = wlslot(rr)
                nc.tensor.matmul(
                    ps0[:, :], OH3[:, rr, :], wl[:, 0:512],
                    start=False, stop=False, skip_group_check=True,
                )
                nc.tensor.matmul(
                    ps1[:, :], OH3[:, rr, :], wl[:, 512:1024],
                    start=False, stop=False, skip_group_check=True,
                ).then_inc(msem, 1)
            rr = RC - 1
            wl = wlslot(rr)
            te.wait_ge(vsem, C[("wlpa", rr)])
            nc.tensor.matmul(
                ps0[:, :], OH3[:, rr, :], wl[:, 0:512],
                start=False, stop=True, skip_group_check=True,
            ).then_inc(msem, 1)
            te.wait_ge(vsem, C[("wlpb", rr)])
            nc.tensor.matmul(
                ps1[:, :], OH3[:, rr, :], wl[:, 512:1024],
                start=False, stop=True, skip_group_check=True,
            ).then_inc(msem, 1)

    return nc


_CACHE = {}


def _buckets(a):
    """Deal r-indices into NK buckets of RC; zeros occupy each bucket's
    fast-mask positions first. Returns (buckets, nfast)."""
    zeros = [r for r in range(R) if a[r] == 0.0]
    pos = [r for r in range(R) if a[r] != 0.0]
    zbuck = [[] for _ in range(NK)]
    for j, r in enumerate(zeros):
        zbuck[j % NK].append(r)
    nfast = min(min(len(zb) for zb in zbuck), RC)
    mask = _fast_mask(nfast)
    pi = 0
    buckets = []
    for k in range(NK):
        zq = list(zbuck[k])
        rl = [None] * RC
        for i in range(RC):
            if mask[i]:
                rl[i] = zq.pop(0)
        for i in range(RC):
            if rl[i] is None:
                if zq:
                    rl[i] = zq.pop(0)
                else:
                    rl[i] = pos[pi]
                    pi += 1
        buckets.append(rl)
    return buckets, nfast


def _prepare(X, T, M, PD, alpha, w_v, w_t, b_t, b_v, ref_time):
    """Pack full inputs into per-core DRAM parameter maps.
    Returns (mask, buckets, in_maps)."""
    a = np.maximum(alpha.reshape(R), 0.0)
    s_ref = ref_time.reshape(R)
    nas = -(a * s_ref)
    bt4 = 4.0 * b_t[..., 0]              # [R, L, D]
    lbv = float(L) * b_v[:, 0, :]        # [R, D]

    buckets, nfast = _buckets(a)
    mask = _fast_mask(nfast)

    # per-r params: [wt0|wt1|wt2|wt3|wv] (5*D per iter)
    wts = np.stack(
        [w_t[..., 0], w_t[..., 1], w_t[..., 2], w_t[..., 3], w_v], axis=2
    )                                     # [R, L, 5, D]

    oh = np.zeros((L, RC, RC), np.float32)
    for r in range(RC):
        oh[:, r, r] = 1.0
    ident = np.eye(L, dtype=np.float32)
    ohd = np.zeros((64, 512), np.float32)
    for b in range(8):
        for d in range(64):
            ohd[d, b * 64 + d] = 1.0

    in_maps = []
    for c in range(8):
        b0 = (c // NK) * BC
        rl = buckets[c % NK]
        tr = lambda x: np.ascontiguousarray(
            x[b0 : b0 + BC].transpose(1, 0, 2).reshape(L, FD)
        )
        cfc = np.zeros((L, CFC_W), np.float32)
        cfc[:, 0:RC] = a[rl]
        cfc[:, RC : 2 * RC] = nas[rl]
        cbf = np.zeros((L, CB_W), np.float32)
        cbf[:, 0:FD] = tr(X)
        cbf[:, FD : 2 * FD] = tr(M)
        cbf[:, 2 * FD : 3 * FD] = tr(PD)
        cbf[:, 3 * FD : 3 * FD + RC * RC] = oh.reshape(L, RC * RC)
        cbf[:, 3 * FD + RC * RC :] = ident
        wp = wts[rl].reshape(RC, L, 5 * D).transpose(1, 0, 2).reshape(L, RC * WS_W)
        btp = np.zeros((64, BT_W), np.float32)
        for i, r in enumerate(rl):
            btp[0:D, i * L : (i + 1) * L] = bt4[r].T
            btp[0:D, RC * L + i] = lbv[r]
        in_maps.append(
            {
                "cfc": cfc,
                "cfT": tr(T).astype(_nbf16),
                "cb": cbf.astype(_nbf16),
                "W": np.ascontiguousarray(wp).astype(_nbf16),
                "BT": btp.astype(_nbf16),
                "OHD": ohd.astype(_nbf16),
            }
        )
    return mask, buckets, in_maps


def kernel(X, T, M, PD, alpha, w_v, w_t, b_t, b_v, ref_time):
    X = np.asarray(X, np.float32)
    T = np.asarray(T, np.float32)
    M = np.asarray(M, np.float32)
    PD = np.asarray(PD, np.float32)
    alpha = np.asarray(alpha, np.float32)
    w_v = np.asarray(w_v, np.float32)
    w_t = np.asarray(w_t, np.float32)
    b_t = np.asarray(b_t, np.float32)
    b_v = np.asarray(b_v, np.float32)
    ref_time = np.asarray(ref_time, np.float32)

    mask, buckets, in_maps = _prepare(
        X, T, M, PD, alpha, w_v, w_t, b_t, b_v, ref_time
    )

    if mask not in _CACHE:
        _CACHE[mask] = _build_graph(mask)
    nc = _CACHE[mask]

    trace = bool(os.environ.get("BASS_KERNEL_TRACE"))
    kw = {}
    if trace:
        tmpdir = os.environ.get("BASS_KERNEL_TRACE_DIR") or None
        kw = dict(trace=True, tmpdir=tmpdir)
    res = run_bass_kernel_spmd(nc, in_maps, core_ids=list(range(8)), **kw)
    if trace:
        _CACHE["exec_time_ns"] = res.exec_time_ns
        print(f"HW exec time: {res.exec_time_ns} ns")

    out = np.zeros((B, R, D), np.float32)
    for c in range(8):
        b0 = (c // NK) * BC
        rl = buckets[c % NK]
        o = np.asarray(res.results[c]["out"], np.float32).reshape(RC, BC, D)
        for i, r in enumerate(rl):
            out[b0 : b0 + BC, r] = o[i]
    return out
